# revision 14
# baseline (speedup 1.0000x reference)
"""Trainium2 Bass kernel for nn_LocalAggregationLoss (retrieval_knn).

loss = mean_b[ log d1_b - log d2_b ]
  d1_b = sum_n bg[b,n] * exp(<v_b, bank_n>/T)
  d2_b = sum_n int[b,n] * exp(<v_b, bank_n>/T)      (int subset of bg)
  v_b = codes_b / ||codes_b||

Sharding: bank and masks sharded over N across 8 cores; per-core partial
sums are all-reduced on the host (as suggested by the sharding hint).

Device layout: [n on partitions, b on free dim].
  - PE: e-logits  psum[n128, b512] = bankT_tile[d,n128].T @ vT[d, b512]
  - ACT: e = exp(psum/T - C)  (f16, C-shift keeps f16 range comfy)
  - combined mask c = bg + 2*int in {0,1,3}:
      s1 = sum e*c   = d1 + 2*d2
      s2 = sum e*c^2 = d1 + 8*d2
    so two DVE multiplies (w1 = e*c, w2 = w1*c) replace the two separate
    mask streams, halving mask DMA traffic.
  - PE ones-matmul reduces w1/w2 over n (partition contraction), col-tiled
    so up to 4 reduce-matmuls run concurrently in separate 32-col strips,
    accumulating into PSUM rows 0/32/64/96 across all tiles.
Host: s1/s2 all-reduce over cores, d2=(s2-s1)/6, d1=(4*s1-s2)/3, loss.
"""

import numpy as np
from contextlib import ExitStack

B, N, D = 512, 100000, 128
TEMP = 0.07
CSHIFT = 6.0
NCORES = 8
N_PAD = 102400  # next multiple of 8*128*2 above N
N_CORE = N_PAD // NCORES  # 12800
P = 128

_CACHE = {}


R = 10  # mask rows per partition per chunk (=> 10KB contiguous DMA descriptors)


def _emit(tc, nc, outs, ins, n_core):
    """Emit the per-core tile program. ins = (bankT, cT, vT); outs = (sacc,).

    n-index mapping (chunk k, partition p, sub-tile j):
      n = k*128*R + p*R + j
    cT in natural row order gives each partition a contiguous R*B f16 run;
    bankT columns are host-permuted so lhsT slices stay contiguous.
    """
    import concourse.bass as bass
    from concourse import mybir

    bankT, cT, vT = ins
    (sacc,) = outs
    f16 = mybir.dt.float16
    f32 = mybir.dt.float32
    ts = bass.ts

    assert n_core % (P * R) == 0
    chunks = n_core // (P * R)
    assert R % 2 == 0
    mac_per_chunk = R // 2

    PR = P * R

    with ExitStack() as ctx:
        const_pool = ctx.enter_context(tc.tile_pool(name="const", bufs=1))
        bank_pool = ctx.enter_context(tc.tile_pool(name="bank", bufs=chunks))
        c_pool = ctx.enter_context(tc.tile_pool(name="cmask", bufs=4))
        e_pool = ctx.enter_context(tc.tile_pool(name="emat", bufs=2))
        w1_pool = ctx.enter_context(tc.tile_pool(name="w1", bufs=3))
        w2_pool = ctx.enter_context(tc.tile_pool(name="w2", bufs=3))
        pe_pool = ctx.enter_context(tc.tile_pool(name="pe", bufs=3, space="PSUM"))
        pacc_pool = ctx.enter_context(tc.tile_pool(name="pacc", bufs=1, space="PSUM"))
        out_pool = ctx.enter_context(tc.tile_pool(name="outp", bufs=1))

        vT_sb = const_pool.tile([D, B], f16)
        nc.sync.dma_start(vT_sb[:], vT)
        ones_sb = const_pool.tile([P, 1], f16)
        nc.vector.memset(ones_sb[:], 1.0)
        scale_sb = const_pool.tile([P, 1], f32)
        nc.vector.memset(scale_sb[:], 1.0 / TEMP)
        bias_sb = const_pool.tile([P, 1], f32)
        nc.vector.memset(bias_sb[:], -CSHIFT)
        # Dummy activation at t~0 so the ~2.7us exp table load overlaps the
        # initial DMAs instead of delaying the first real EXP.
        warm_sb = const_pool.tile([P, 1], f32)
        nc.scalar.activation(
            warm_sb[:], scale_sb[:], mybir.ActivationFunctionType.Exp
        )

        # cT dram [n_core, B] natural order == [chunks, P, R, B]; partition p
        # of chunk k holds rows k*P*R + p*R + j, a contiguous R*B run.
        cT_r = cT.rearrange("(k p j) b -> p k j b", p=P, j=R)

        # Per-chunk bank slabs on gpsimd-issued DMAs (descriptor-gen runs in
        # parallel with the sync-engine c DMAs); compute on chunk k only
        # waits for its own slab. First few c DMAs interleave so their
        # descriptors aren't queued behind the whole bank.
        bank_t = []
        c_t = {}
        for k in range(chunks):
            bt = bank_pool.tile([D, PR], f16, name="bank_t", tag="bank_t")
            nc.gpsimd.dma_start(bt[:], bankT[:, k * PR : (k + 1) * PR])
            bank_t.append(bt)
            if k < 3:
                c_t[k] = c_pool.tile([P, R, B], f16, name="c_t", tag="c_t")
                nc.sync.dma_start(c_t[k][:, 0 : R // 2, :], cT_r[:, k, 0 : R // 2, :])
                nc.sync.dma_start(c_t[k][:, R // 2 : R, :], cT_r[:, k, R // 2 : R, :])

        psum_acc = pacc_pool.tile([P, B], f32)
        # Reduce-matmuls only touch rows 0/32/64/96; zero the rest so the
        # full-tile copy at the end reads initialized memory.
        nc.vector.memset(psum_acc[:], 0.0)

        def emit_reduces(k, w1, w2):
            # M=1 reductions over the partition (n) axis, rotated over the
            # four 32-col strips of the PE array so they stream concurrently.
            first = k == 0
            last = k == chunks - 1
            for j in range(R):
                for h, w in enumerate((w1, w2)):
                    strip = (2 * j + h) % 4
                    nc.tensor.matmul(
                        psum_acc[32 * strip : 32 * strip + 1, :],
                        lhsT=ones_sb[:],
                        rhs=w[:, j, :],
                        start=first and j < 2,
                        stop=last and j >= R - 2,
                        tile_position=(0, 32 * strip),
                        skip_group_check=True,
                    )

        pending = []  # (k, w1, w2) reduces delayed TWO chunks: PE stream is
        # [e-MMs k][reduces k-2], and mult2(k-2) finished during chunk k-1,
        # so reduces never stall the PE and the exp feed stays dense.
        last = chunks - 1
        for k in range(chunks):
            if k not in c_t:
                c_t[k] = c_pool.tile([P, R, B], f16, name="c_t", tag="c_t")
                nc.sync.dma_start(c_t[k][:, 0 : R // 2, :], cT_r[:, k, 0 : R // 2, :])
                nc.sync.dma_start(c_t[k][:, R // 2 : R, :], cT_r[:, k, R // 2 : R, :])

            e_t = e_pool.tile([P, R, B], f16)
            for m in range(mac_per_chunk):
                j0 = 2 * m
                pe = pe_pool.tile([P, 2, B], f32)
                nc.tensor.matmul(
                    pe[:, 0, :], lhsT=bank_t[k][:, ts(j0, P)], rhs=vT_sb[:],
                    start=True, stop=True,
                )
                nc.tensor.matmul(
                    pe[:, 1, :], lhsT=bank_t[k][:, ts(j0 + 1, P)], rhs=vT_sb[:],
                    start=True, stop=True,
                )
                nc.scalar.activation(
                    e_t[:, j0 : j0 + 2, :], pe[:], mybir.ActivationFunctionType.Exp,
                    bias=bias_sb[:], scale=scale_sb[:],
                )

            while len(pending) >= 2:
                emit_reduces(*pending.pop(0))

            if k < last:
                # Half-chunk multiplies: same DVE throughput (2x mode), but
                # the first pair only waits for ~half the chunk's EXPs, so
                # the DVE stream starts and drains earlier.
                w1 = w1_pool.tile([P, R, B], f16)
                w2 = w2_pool.tile([P, R, B], f16)
                H = R // 2
                w2_eng = nc.gpsimd if k % 2 == 0 else nc.vector
                for h in (0, 1):
                    sl = (slice(None), slice(h * H, (h + 1) * H), slice(None))
                    nc.vector.tensor_tensor(
                        out=w1[sl], in0=e_t[sl], in1=c_t[k][sl],
                        op=mybir.AluOpType.mult,
                    )
                    w2_eng.tensor_tensor(
                        out=w2[sl], in0=w1[sl], in1=c_t[k][sl],
                        op=mybir.AluOpType.mult,
                    )
                pending.append((k, w1, w2))
            else:
                # Last chunk: drain older reduces, then per-macro multiplies
                # with inline reduces so the tail pipeline stays short.
                while pending:
                    emit_reduces(*pending.pop(0))
                w1 = w1_pool.tile([P, R, B], f16)
                w2 = w2_pool.tile([P, R, B], f16)
                for m in range(mac_per_chunk):
                    j0 = 2 * m
                    sl = (slice(None), slice(j0, j0 + 2), slice(None))
                    nc.vector.tensor_tensor(
                        out=w1[sl], in0=e_t[sl], in1=c_t[k][sl],
                        op=mybir.AluOpType.mult,
                    )
                    nc.vector.tensor_tensor(
                        out=w2[sl], in0=w1[sl], in1=c_t[k][sl],
                        op=mybir.AluOpType.mult,
                    )
                    for j in (j0, j0 + 1):
                        for h, w in enumerate((w1, w2)):
                            strip = (2 * j + h) % 4
                            nc.tensor.matmul(
                                psum_acc[32 * strip : 32 * strip + 1, :],
                                lhsT=ones_sb[:],
                                rhs=w[:, j, :],
                                start=False,
                                stop=m == mac_per_chunk - 1,
                                tile_position=(0, 32 * strip),
                                skip_group_check=True,
                            )
            del c_t[k]

        acc_sb = out_pool.tile([P, B], f32)
        nc.vector.tensor_copy(out=acc_sb[:], in_=psum_acc[:])
        nc.sync.dma_start(sacc, acc_sb[:])


def _build(n_core):
    key = ("prog", n_core)
    if key in _CACHE:
        return _CACHE[key]
    import concourse.bass as bass
    import concourse.tile as tile
    from concourse import bacc, mybir

    nc = bacc.Bacc("TRN2", target_bir_lowering=False, debug=False, num_devices=NCORES)
    f16 = mybir.dt.float16
    f32 = mybir.dt.float32
    bankT = nc.dram_tensor("bankT", [D, n_core], f16, kind="ExternalInput").ap()
    cT = nc.dram_tensor("cT", [n_core, B], f16, kind="ExternalInput").ap()
    vT = nc.dram_tensor("vT", [D, B], f16, kind="ExternalInput").ap()
    sacc = nc.dram_tensor("sacc", [P, B], f32, kind="ExternalOutput").ap()

    with tile.TileContext(nc) as tc:
        _emit(tc, nc, (sacc,), (bankT, cT, vT), n_core)

    nc.compile()
    _CACHE[key] = nc
    return nc


def _host_prep(codes, bank, bg_mask, int_mask):
    """Shard + dtype/layout conversion. Returns per-core input maps."""
    codes = np.asarray(codes, dtype=np.float32)
    bank = np.asarray(bank, dtype=np.float32)
    bg = np.asarray(bg_mask)
    it = np.asarray(int_mask)

    # v = codes / ||codes||, laid out [D, B] for the matmul moving operand
    norms = np.sqrt((codes.astype(np.float64) ** 2).sum(axis=1))
    vT = np.ascontiguousarray((codes / norms[:, None].astype(np.float32)).T).astype(
        np.float16
    )

    # bank transposed [D, N] and zero-padded to N_PAD along n
    bankT = np.zeros((D, N_PAD), dtype=np.float16)
    bankT[:, :N] = bank.T.astype(np.float16)
    # Permute columns within each P*R chunk so device lhsT slices are
    # contiguous under the n = k*P*R + p*R + j mapping (cT stays natural):
    # bankT_il[:, k*P*R + j*P + p] = bankT[:, k*P*R + p*R + j]
    bankT = np.ascontiguousarray(
        bankT.reshape(D, N_PAD // (P * R), P, R).transpose(0, 1, 3, 2).reshape(D, N_PAD)
    )

    # combined mask c = bg + 2*int, transposed to [N_PAD, B]
    cT = np.zeros((N_PAD, B), dtype=np.float16)
    cu8 = bg.astype(np.uint8) + (it.astype(np.uint8) << 1)
    cT[:N, :] = cu8.T.astype(np.float16)

    in_maps = []
    for c in range(NCORES):
        lo, hi = c * N_CORE, (c + 1) * N_CORE
        in_maps.append(
            {
                "bankT": np.ascontiguousarray(bankT[:, lo:hi]),
                "cT": np.ascontiguousarray(cT[lo:hi, :]),
                "vT": vT,
            }
        )
    return in_maps


def _host_reduce(results):
    s1 = np.zeros(B, dtype=np.float64)
    s2 = np.zeros(B, dtype=np.float64)
    for r in results:
        sacc = r["sacc"].astype(np.float64)
        s1 += sacc[0] + sacc[64]
        s2 += sacc[32] + sacc[96]
    d2 = (s2 - s1) / 6.0
    d1 = (4.0 * s1 - s2) / 3.0
    loss = np.mean(np.log(d1) - np.log(d2))
    return np.float32(loss)


def kernel(codes, bank, bg_mask, int_mask, _trace=False):
    from concourse.bass_utils import run_bass_kernel_spmd

    nc = _build(N_CORE)
    in_maps = _host_prep(codes, bank, bg_mask, int_mask)
    res = run_bass_kernel_spmd(nc, in_maps, core_ids=list(range(NCORES)), trace=_trace)
    out = _host_reduce(res.results)
    if _trace:
        return out, res
    return out


# revision 16
# speedup vs baseline: 1.3805x; 1.3805x over previous
"""Trainium2 Bass kernel for nn_LocalAggregationLoss (retrieval_knn).

loss = mean_b[ log d1_b - log d2_b ]
  d1_b = sum_n bg[b,n] * exp(<v_b, bank_n>/T)
  d2_b = sum_n int[b,n] * exp(<v_b, bank_n>/T)      (int subset of bg)
  v_b = codes_b / ||codes_b||

Sharding: bank and masks sharded over N across 8 cores; per-core partial
sums are all-reduced on the host (as suggested by the sharding hint).

Device layout: [n on partitions, b on free dim].
  - PE: e-logits  psum[n128, b512] = bankT_tile[d,n128].T @ vT[d, b512]
  - ACT: e = exp(psum/T - C)  (f16, C-shift keeps f16 range comfy)
  - combined mask c = bg + 2*int in {0,1,3}:
      s1 = sum e*c   = d1 + 2*d2
      s2 = sum e*c^2 = d1 + 8*d2
    so two DVE multiplies (w1 = e*c, w2 = w1*c) replace the two separate
    mask streams, halving mask DMA traffic.
  - PE ones-matmul reduces w1/w2 over n (partition contraction), col-tiled
    so up to 4 reduce-matmuls run concurrently in separate 32-col strips,
    accumulating into PSUM rows 0/32/64/96 across all tiles.
Host: s1/s2 all-reduce over cores, d2=(s2-s1)/6, d1=(4*s1-s2)/3, loss.
"""

import numpy as np
from contextlib import ExitStack

B, N, D = 512, 100000, 128
TEMP = 0.07
CSHIFT = 6.0
NCORES = 8
N_PAD = 102400  # next multiple of 8*128*2 above N
N_CORE = N_PAD // NCORES  # 12800
P = 128

_CACHE = {}


R = 10  # mask rows per partition per chunk (=> 10KB contiguous DMA descriptors)


def _emit(tc, nc, outs, ins, n_core):
    """Emit the per-core tile program. ins = (bankT, cT, vT); outs = (sacc,).

    n-index mapping (chunk k, partition p, sub-tile j):
      n = k*128*R + p*R + j
    cT in natural row order gives each partition a contiguous R*B f16 run;
    bankT columns are host-permuted so lhsT slices stay contiguous.
    """
    import concourse.bass as bass
    from concourse import mybir

    bankT, cT, vT = ins
    (sacc,) = outs
    f16 = mybir.dt.float16
    f32 = mybir.dt.float32
    ts = bass.ts

    assert n_core % (P * R) == 0
    chunks = n_core // (P * R)
    assert R % 2 == 0
    mac_per_chunk = R // 2

    PR = P * R

    with ExitStack() as ctx:
        const_pool = ctx.enter_context(tc.tile_pool(name="const", bufs=1))
        bank_pool = ctx.enter_context(tc.tile_pool(name="bank", bufs=chunks))
        c_pool = ctx.enter_context(tc.tile_pool(name="cmask", bufs=4))
        e_pool = ctx.enter_context(tc.tile_pool(name="emat", bufs=2))
        w1_pool = ctx.enter_context(tc.tile_pool(name="w1", bufs=3))
        w2_pool = ctx.enter_context(tc.tile_pool(name="w2", bufs=3))
        pe_pool = ctx.enter_context(tc.tile_pool(name="pe", bufs=3, space="PSUM"))
        pacc_pool = ctx.enter_context(tc.tile_pool(name="pacc", bufs=1, space="PSUM"))
        out_pool = ctx.enter_context(tc.tile_pool(name="outp", bufs=1))

        vT_sb = const_pool.tile([D, B], f16)
        nc.sync.dma_start(vT_sb[:], vT)
        ones_sb = const_pool.tile([P, 1], f16)
        nc.vector.memset(ones_sb[:], 1.0)
        scale_sb = const_pool.tile([P, 1], f32)
        nc.vector.memset(scale_sb[:], 1.0 / TEMP)
        bias_sb = const_pool.tile([P, 1], f32)
        nc.vector.memset(bias_sb[:], -CSHIFT)
        # Dummy activation at t~0 so the ~2.7us exp table load overlaps the
        # initial DMAs instead of delaying the first real EXP.
        warm_sb = const_pool.tile([P, 1], f32)
        nc.scalar.activation(
            warm_sb[:], scale_sb[:], mybir.ActivationFunctionType.Exp
        )

        # cT dram [n_core, B] natural order == [chunks, P, R, B]; partition p
        # of chunk k holds rows k*P*R + p*R + j, a contiguous R*B run.
        cT_r = cT.rearrange("(k p j) b -> p k j b", p=P, j=R)

        # Per-chunk bank slabs on gpsimd-issued DMAs (descriptor-gen runs in
        # parallel with the sync-engine c DMAs); compute on chunk k only
        # waits for its own slab. First few c DMAs interleave so their
        # descriptors aren't queued behind the whole bank.
        bank_t = []
        c_t = {}
        for k in range(chunks):
            bt = bank_pool.tile([D, PR], f16, name="bank_t", tag="bank_t")
            nc.gpsimd.dma_start(bt[:], bankT[:, k * PR : (k + 1) * PR])
            bank_t.append(bt)
            if k < 3:
                c_t[k] = c_pool.tile([P, R, B], f16, name="c_t", tag="c_t")
                nc.sync.dma_start(c_t[k][:, 0 : R // 2, :], cT_r[:, k, 0 : R // 2, :])
                nc.sync.dma_start(c_t[k][:, R // 2 : R, :], cT_r[:, k, R // 2 : R, :])

        psum_acc = pacc_pool.tile([P, B], f32)
        # Reduce-matmuls only touch rows 0/32/64/96; zero the rest so the
        # full-tile copy at the end reads initialized memory.
        nc.vector.memset(psum_acc[:], 0.0)

        def emit_reduces(k, w1, w2):
            # M=1 reductions over the partition (n) axis, rotated over the
            # four 32-col strips of the PE array so they stream concurrently.
            first = k == 0
            last = k == chunks - 1
            for j in range(R):
                for h, w in enumerate((w1, w2)):
                    strip = (2 * j + h) % 4
                    nc.tensor.matmul(
                        psum_acc[32 * strip : 32 * strip + 1, :],
                        lhsT=ones_sb[:],
                        rhs=w[:, j, :],
                        start=first and j < 2,
                        stop=last and j >= R - 2,
                        tile_position=(0, 32 * strip),
                        skip_group_check=True,
                    )

        pending = []  # (k, w1, w2) reduces delayed TWO chunks: PE stream is
        # [e-MMs k][reduces k-2], and mult2(k-2) finished during chunk k-1,
        # so reduces never stall the PE and the exp feed stays dense.
        last = chunks - 1
        for k in range(chunks):
            if k not in c_t:
                c_t[k] = c_pool.tile([P, R, B], f16, name="c_t", tag="c_t")
                nc.sync.dma_start(c_t[k][:, 0 : R // 2, :], cT_r[:, k, 0 : R // 2, :])
                nc.sync.dma_start(c_t[k][:, R // 2 : R, :], cT_r[:, k, R // 2 : R, :])

            e_t = e_pool.tile([P, R, B], f16)
            for m in range(mac_per_chunk):
                j0 = 2 * m
                pe = pe_pool.tile([P, 2, B], f32)
                nc.tensor.matmul(
                    pe[:, 0, :], lhsT=bank_t[k][:, ts(j0, P)], rhs=vT_sb[:],
                    start=True, stop=True,
                )
                nc.tensor.matmul(
                    pe[:, 1, :], lhsT=bank_t[k][:, ts(j0 + 1, P)], rhs=vT_sb[:],
                    start=True, stop=True,
                )
                nc.scalar.activation(
                    e_t[:, j0 : j0 + 2, :], pe[:], mybir.ActivationFunctionType.Exp,
                    bias=bias_sb[:], scale=scale_sb[:],
                )

            while len(pending) >= 2:
                emit_reduces(*pending.pop(0))

            if k < last:
                # Sub-chunk multiplies: same DVE throughput (2x mode), but
                # the first pair only waits for part of the chunk's EXPs, so
                # the DVE stream starts and drains earlier. Chunk 0 goes
                # per-macro so the DVE starts right after the first EXP.
                w1 = w1_pool.tile([P, R, B], f16)
                w2 = w2_pool.tile([P, R, B], f16)
                step = 2 if k == 0 else R // 2
                for j0 in range(0, R, step):
                    sl = (slice(None), slice(j0, j0 + step), slice(None))
                    nc.vector.tensor_tensor(
                        out=w1[sl], in0=e_t[sl], in1=c_t[k][sl],
                        op=mybir.AluOpType.mult,
                    )
                    nc.vector.tensor_tensor(
                        out=w2[sl], in0=w1[sl], in1=c_t[k][sl],
                        op=mybir.AluOpType.mult,
                    )
                pending.append((k, w1, w2))
            else:
                # Last chunk: drain older reduces, then per-macro multiplies
                # with inline reduces so the tail pipeline stays short.
                while pending:
                    emit_reduces(*pending.pop(0))
                w1 = w1_pool.tile([P, R, B], f16)
                w2 = w2_pool.tile([P, R, B], f16)
                for m in range(mac_per_chunk):
                    j0 = 2 * m
                    sl = (slice(None), slice(j0, j0 + 2), slice(None))
                    nc.vector.tensor_tensor(
                        out=w1[sl], in0=e_t[sl], in1=c_t[k][sl],
                        op=mybir.AluOpType.mult,
                    )
                    nc.vector.tensor_tensor(
                        out=w2[sl], in0=w1[sl], in1=c_t[k][sl],
                        op=mybir.AluOpType.mult,
                    )
                    for j in (j0, j0 + 1):
                        for h, w in enumerate((w1, w2)):
                            strip = (2 * j + h) % 4
                            nc.tensor.matmul(
                                psum_acc[32 * strip : 32 * strip + 1, :],
                                lhsT=ones_sb[:],
                                rhs=w[:, j, :],
                                start=False,
                                stop=m == mac_per_chunk - 1,
                                tile_position=(0, 32 * strip),
                                skip_group_check=True,
                            )
            del c_t[k]

        acc_sb = out_pool.tile([P, B], f32)
        nc.vector.tensor_copy(out=acc_sb[:], in_=psum_acc[:])
        nc.sync.dma_start(sacc, acc_sb[:])


def _build(n_core):
    key = ("prog", n_core)
    if key in _CACHE:
        return _CACHE[key]
    import concourse.bass as bass
    import concourse.tile as tile
    from concourse import bacc, mybir

    nc = bacc.Bacc("TRN2", target_bir_lowering=False, debug=False, num_devices=NCORES)
    f16 = mybir.dt.float16
    f32 = mybir.dt.float32
    bankT = nc.dram_tensor("bankT", [D, n_core], f16, kind="ExternalInput").ap()
    cT = nc.dram_tensor("cT", [n_core, B], f16, kind="ExternalInput").ap()
    vT = nc.dram_tensor("vT", [D, B], f16, kind="ExternalInput").ap()
    sacc = nc.dram_tensor("sacc", [P, B], f32, kind="ExternalOutput").ap()

    with tile.TileContext(nc) as tc:
        _emit(tc, nc, (sacc,), (bankT, cT, vT), n_core)

    nc.compile()
    _CACHE[key] = nc
    return nc


def _host_prep(codes, bank, bg_mask, int_mask):
    """Shard + dtype/layout conversion. Returns per-core input maps."""
    codes = np.asarray(codes, dtype=np.float32)
    bank = np.asarray(bank, dtype=np.float32)
    bg = np.asarray(bg_mask)
    it = np.asarray(int_mask)

    # v = codes / ||codes||, laid out [D, B] for the matmul moving operand
    norms = np.sqrt((codes.astype(np.float64) ** 2).sum(axis=1))
    vT = np.ascontiguousarray((codes / norms[:, None].astype(np.float32)).T).astype(
        np.float16
    )

    # bank transposed [D, N] and zero-padded to N_PAD along n
    bankT = np.zeros((D, N_PAD), dtype=np.float16)
    bankT[:, :N] = bank.T.astype(np.float16)
    # Permute columns within each P*R chunk so device lhsT slices are
    # contiguous under the n = k*P*R + p*R + j mapping (cT stays natural):
    # bankT_il[:, k*P*R + j*P + p] = bankT[:, k*P*R + p*R + j]
    bankT = np.ascontiguousarray(
        bankT.reshape(D, N_PAD // (P * R), P, R).transpose(0, 1, 3, 2).reshape(D, N_PAD)
    )

    # combined mask c = bg + 2*int, transposed to [N_PAD, B]
    cT = np.zeros((N_PAD, B), dtype=np.float16)
    cu8 = bg.astype(np.uint8) + (it.astype(np.uint8) << 1)
    cT[:N, :] = cu8.T.astype(np.float16)

    in_maps = []
    for c in range(NCORES):
        lo, hi = c * N_CORE, (c + 1) * N_CORE
        in_maps.append(
            {
                "bankT": np.ascontiguousarray(bankT[:, lo:hi]),
                "cT": np.ascontiguousarray(cT[lo:hi, :]),
                "vT": vT,
            }
        )
    return in_maps


def _host_reduce(results):
    s1 = np.zeros(B, dtype=np.float64)
    s2 = np.zeros(B, dtype=np.float64)
    for r in results:
        sacc = r["sacc"].astype(np.float64)
        s1 += sacc[0] + sacc[64]
        s2 += sacc[32] + sacc[96]
    d2 = (s2 - s1) / 6.0
    d1 = (4.0 * s1 - s2) / 3.0
    loss = np.mean(np.log(d1) - np.log(d2))
    return np.float32(loss)


def kernel(codes, bank, bg_mask, int_mask, _trace=False):
    from concourse.bass_utils import run_bass_kernel_spmd

    nc = _build(N_CORE)
    in_maps = _host_prep(codes, bank, bg_mask, int_mask)
    res = run_bass_kernel_spmd(nc, in_maps, core_ids=list(range(NCORES)), trace=_trace)
    out = _host_reduce(res.results)
    if _trace:
        return out, res
    return out
